# revision 21
# baseline (speedup 1.0000x reference)
"""Trainium2 Bass kernel for nn_Block_Height_Reducing_Filtering.

Computes, per head n and batch b:
    sT[h,d]  = x_b^T @ W_pool[n]^T          (pooling 1x1 conv, pre-softmax)
    ET       = exp(sT)                       (unnormalized softmax over h)
    ht[d,c]  = (ET^T @ x_b^T) / Z[d]         (softmax-pooled, [d,C] layout)
    y[e,c]   = W_pw[n] @ ht                  (pointwise conv)
    y2       = ht + relu(BN(y))              (BN stats over (B,C), global)
    m[o,dd]  = W_merge @ concat_n(y2^T)      (merge conv)
    out      = relu(BN(m))                   (BN stats over (B,d), global)

Sharding: data-parallel over batch B=16 across 8 cores (2 batches each).
BN batch statistics are made exact via small AllReduces: one 4KB AllReduce
per head (pipelined against the next head's compute) plus one 1KB
AllReduce for the merge BN. Matmuls run in bf16 (fp32 PSUM accumulation);
softmax/BN math in fp32. Host-side packing pads h (2000 = 16x125) and
d/e (500) to 128-multiples so every matmul uses full 128-row weights.
"""

import functools
import os
import sys

import numpy as np

if "/opt/trn_rl_repo" not in sys.path:
    sys.path.insert(0, "/opt/trn_rl_repo")

import ml_dtypes

BF = ml_dtypes.bfloat16

N_CORES = 8
B, C, H, D, NH, CO = 16, 128, 2000, 500, 4, 128
HB = 16          # h blocks
HBS = 125        # real rows per h block
DP = 512         # padded d / e
EPS = 1e-5


@functools.lru_cache(maxsize=1)
def _build():
    from concourse import bacc, mybir
    from concourse.tile import TileContext

    f32 = mybir.dt.float32
    bf16 = mybir.dt.bfloat16
    Alu = mybir.AluOpType
    Act = mybir.ActivationFunctionType

    SQ_DVE = True   # squared-sum stats on DVE (scalar_tensor_tensor)
    BF16HT = True   # ht kept bf16-only (validated: rel err 0.0047)

    nc = bacc.Bacc(
        "TRN2",
        target_bir_lowering=False,
        debug=False,
        enable_asserts=False,
        num_devices=N_CORES,
    )

    # -------- kernel I/O (all pre-packed on host into [128, F] layouts) ----
    xpk = nc.dram_tensor("xpk", [128, 2 * HB * 128], bf16, kind="ExternalInput").ap()
    xtg = nc.dram_tensor("xtg", [128, 2 * HB * 129], bf16, kind="ExternalInput").ap()
    wpt = nc.dram_tensor("wpt", [128, NH * DP], bf16, kind="ExternalInput").ap()
    wpwt = nc.dram_tensor("wpwt", [128, NH * 4 * DP], bf16, kind="ExternalInput").ap()
    wmt = nc.dram_tensor("wmt", [128, NH * 128], bf16, kind="ExternalInput").ap()
    gpw = nc.dram_tensor("gpw", [128, 16], f32, kind="ExternalInput").ap()
    bpw = nc.dram_tensor("bpw", [128, 16], f32, kind="ExternalInput").ap()
    gmm = nc.dram_tensor("gmm", [128, 1], f32, kind="ExternalInput").ap()
    bmm = nc.dram_tensor("bmm", [128, 1], f32, kind="ExternalInput").ap()
    idn = nc.dram_tensor("idn", [128, 128], bf16, kind="ExternalInput").ap()
    out = nc.dram_tensor("out", [128, 2 * D], f32, kind="ExternalOutput").ap()

    with TileContext(nc) as tc:
        with (
            tc.tile_pool(name="const", bufs=1) as const,
            tc.tile_pool(name="work", bufs=2) as work,
            tc.tile_pool(name="ps", bufs=2, space="PSUM") as ps,
            tc.tile_pool(name="psht", bufs=4, space="PSUM") as psht,
            tc.tile_pool(name="dram", bufs=1, space="DRAM") as dram,
        ):
            # ------------- resident inputs (compute-critical first) --------
            xpk_sb = const.tile([128, 2 * HB * 128], bf16)
            wpt_sb = const.tile([128, NH * DP], bf16)
            xtg_sb = const.tile([128, 2 * HB * 129], bf16)
            wpwt_sb = const.tile([128, NH * 4 * DP], bf16)
            wmt_sb = const.tile([128, NH * 128], bf16)
            gpw_sb = const.tile([128, 16], f32)
            bpw_sb = const.tile([128, 16], f32)
            gmm_sb = const.tile([128, 1], f32)
            bmm_sb = const.tile([128, 1], f32)
            idn_sb = const.tile([128, 128], bf16)
            def dma_parts(eng, sb, dr, parts):
                stp = sb.shape[1] // parts
                for i in range(parts):
                    eng.dma_start(sb[:, i * stp:(i + 1) * stp],
                                  dr[:, i * stp:(i + 1) * stp])
            # first head's working set first, on the Sync queue
            nc.sync.dma_start(xpk_sb[:, 0:1024], xpk[:, 0:1024])
            nc.sync.dma_start(wpt_sb[:, 0:DP], wpt[:, 0:DP])
            dma_parts(nc.sync, xtg_sb, xtg, 4)
            nc.sync.dma_start(xpk_sb[:, 1024:4096], xpk[:, 1024:4096])
            nc.sync.dma_start(wpt_sb[:, DP:], wpt[:, DP:])
            # bulk weights on the gpsimd queue (idle at this point)
            dma_parts(nc.gpsimd, wpwt_sb, wpwt, 2)
            for sb, dr in (
                (wmt_sb, wmt), (gpw_sb, gpw), (bpw_sb, bpw), (gmm_sb, gmm),
                (bmm_sb, bmm), (idn_sb, idn),
            ):
                nc.gpsimd.dma_start(sb[:], dr[:])

            # ------------- persistent intermediates -------------
            htn_bf = const.tile([128, 16 * 256], bf16)    # ht normalized, bf16
            htn_f32 = None
            if not BF16HT:
                htn_f32 = const.tile([128, 16 * 256], f32)
            y_all = const.tile([128, 16 * 256], f32)      # pointwise conv out
            head2_sb = const.tile([128, 2 * NH * DP], bf16)  # y2^T per b, n-major
            m_sb = const.tile([128, 2 * D], f32)          # merge conv out per b
            mstat = const.tile([128, 4], f32)             # merge sums per b
            mstat2 = const.tile([128, 2], f32)            # combined merge stats

            N_ARS = 4  # one stats AllReduce per head, pipelined
            if N_ARS == 4:
                stats_n = [const.tile([128, 8], f32, name=f"stats_{n}")
                           for n in range(NH)]
                statsg_n = [const.tile([128, 8], f32, name=f"statsg_{n}")
                            for n in range(NH)]
            else:
                stats_all = const.tile([128, 32], f32, name="stats_all")
                statsg_all = const.tile([128, 32], f32, name="statsg_all")
                stats_n = [stats_all[:, n * 8:(n + 1) * 8] for n in range(NH)]
                statsg_n = [statsg_all[:, n * 8:(n + 1) * 8]
                            for n in range(NH)]

            rg = [list(range(N_CORES))]

            # ================= stage A: pooling + pointwise (one head) =====
            def stage_a(n):
                for b in range(2):
                    ht_ps = []
                    for dc in range(4):
                        t = psht.tile([128, 129], f32, tag="ht",
                                      name=f"htps_{n}_{b}_{dc}")
                        ht_ps.append(t)
                    def mm1pair(p):
                        # two h-chunks share one 2-bank PSUM tensor and one exp
                        s_ps = ps.tile([128, 2 * DP], f32, tag="mm",
                                       name=f"sps_{n}_{b}_{p}")
                        for j in range(2):
                            k = 2 * p + j
                            nc.tensor.matmul(
                                s_ps[:, j * DP:(j + 1) * DP],
                                lhsT=xpk_sb[:, b * 2048 + k * 128:
                                            b * 2048 + (k + 1) * 128],
                                rhs=wpt_sb[:, n * DP:(n + 1) * DP],
                                start=True, stop=True,
                            )
                        et = work.tile([128, 2 * DP], bf16, tag="et", bufs=3,
                                       name=f"et_{n}_{b}_{p}")
                        nc.scalar.activation(et[:], s_ps[:], Act.Exp)
                        return et

                    def mm2pair(p, et):
                        for j in range(2):
                            k = 2 * p + j
                            rhs_x = xtg_sb[:, (b * HB + k) * 129:
                                           (b * HB + k + 1) * 129]
                            for dc in range(4):
                                nc.tensor.matmul(
                                    ht_ps[dc][:],
                                    lhsT=et[:, j * DP + dc * 128:
                                            j * DP + (dc + 1) * 128],
                                    rhs=rhs_x,
                                    start=(k == 0), stop=(k == HB - 1),
                                )

                    et_prev = mm1pair(0)
                    for p in range(1, HB // 2):
                        et_next = mm1pair(p)
                        mm2pair(p - 1, et_prev)
                        et_prev = et_next
                    mm2pair(HB // 2 - 1, et_prev)
                    for dc in range(4):
                        zinv = work.tile([128, 1], f32, tag="zinv", bufs=4,
                                         name=f"zinv_{n}_{b}_{dc}")
                        nc.vector.reciprocal(zinv[:], ht_ps[dc][:, 128:129])
                        off = (n * 4 + dc) * 256 + b * 128
                        nc.vector.tensor_scalar_mul(
                            htn_bf[:, off:off + 128], ht_ps[dc][:, 0:128],
                            zinv[:])
                        if not BF16HT:
                            nc.vector.tensor_scalar_mul(
                                htn_f32[:, off:off + 128], ht_ps[dc][:, 0:128],
                                zinv[:])
                # pointwise conv for head n (both batches, N=256)
                for ec in range(4):
                    y_ps = psht.tile([128, 256], f32, tag="ht",
                                    name=f"yps_{n}_{ec}")
                    for kc in range(4):
                        nc.tensor.matmul(
                            y_ps[:],
                            lhsT=wpwt_sb[:, (n * 4 + kc) * DP + ec * 128:
                                         (n * 4 + kc) * DP + (ec + 1) * 128],
                            rhs=htn_bf[:, (n * 4 + kc) * 256:
                                       (n * 4 + kc + 1) * 256],
                            start=(kc == 0), stop=(kc == 3),
                        )
                    col = n * 4 + ec
                    nc.vector.tensor_scalar(
                        y_all[:, col * 256:(col + 1) * 256], y_ps[:],
                        1.0, None, Alu.mult, op1=Alu.add,
                        accum_out=stats_n[n][:, ec:ec + 1],
                    )
                    sq = work.tile([128, 256], f32, tag="sq",
                                   name=f"sqa_{n}_{ec}")
                    if SQ_DVE:
                        ysb = y_all[:, col * 256:(col + 1) * 256]
                        nc.vector.scalar_tensor_tensor(
                            out=sq[:], in0=y_ps[:], scalar=1.0, in1=ysb,
                            op0=Alu.mult, op1=Alu.mult,
                            accum_out=stats_n[n][:, 4 + ec:5 + ec],
                        )
                    else:
                        nc.scalar.activation(
                            sq[:], y_ps[:], Act.Square,
                            accum_out=stats_n[n][:, 4 + ec:5 + ec],
                        )

            # ============== per-head stats AllReduce ======================
            if N_ARS == 4:
                ar_in = [dram.tile([128, 8], f32, name=f"ar_in_{n}")
                         for n in range(NH)]
                ar_out = [dram.tile([128, 8], f32, addr_space="Shared",
                                    name=f"ar_out_{n}") for n in range(NH)]

                def stage_ar(n):
                    nc.sync.dma_start(ar_in[n][:], stats_n[n][:])
                    nc.gpsimd.collective_compute(
                        "AllReduce", Alu.add, replica_groups=rg,
                        ins=[ar_in[n][:].opt()], outs=[ar_out[n][:].opt()],
                    )
                    nc.gpsimd.dma_start(statsg_n[n][:], ar_out[n][:])
            else:
                ar_in1 = dram.tile([128, 32], f32, name="ar_in1")
                ar_out1 = dram.tile([128, 32], f32, addr_space="Shared",
                                    name="ar_out1")

                def stage_ar(n):
                    if n != NH - 1:
                        return
                    nc.sync.dma_start(ar_in1[:], stats_all[:])
                    nc.gpsimd.collective_compute(
                        "AllReduce", Alu.add, replica_groups=rg,
                        ins=[ar_in1[:].opt()], outs=[ar_out1[:].opt()],
                    )
                    nc.gpsimd.dma_start(statsg_all[:], ar_out1[:])

            # ===== BN coefficient helper (per-partition scale/bias) ========
            def bn_coeffs(sum_ap, sq_ap, inv_n, g_ap, b_ap, scr, w):
                # vectorized over w columns; returns (scale, bias) [128, w]
                mean = scr[:, 0:w]
                ex2 = scr[:, w:2 * w]
                var = scr[:, 2 * w:3 * w]
                rstd = scr[:, 3 * w:4 * w]
                sc = scr[:, 4 * w:5 * w]
                bi = scr[:, 5 * w:6 * w]
                std = scr[:, 6 * w:7 * w]
                nc.vector.tensor_scalar_mul(mean, sum_ap, inv_n)
                nc.vector.tensor_scalar_mul(ex2, sq_ap, inv_n)
                for j in range(w):  # mean^2 via per-partition scalar port
                    nc.vector.tensor_scalar(
                        var[:, j:j + 1], mean[:, j:j + 1],
                        mean[:, j:j + 1], None, Alu.mult)
                nc.vector.tensor_sub(var, ex2, var)
                nc.vector.tensor_scalar_add(var, var, EPS)
                nc.scalar.sqrt(std, var)
                nc.vector.reciprocal(rstd, std)
                nc.vector.tensor_mul(sc, rstd, g_ap)
                nc.vector.tensor_mul(bi, mean, sc)
                nc.vector.tensor_sub(bi, b_ap, bi)
                return sc, bi

            # ===== stage C: BN apply + residual + transpose (one head) =====
            def stage_c(n):
                scr = work.tile([128, 32], f32, tag="bnscr", bufs=2,
                                name=f"bnscr_{n}")
                sc4, bi4 = bn_coeffs(
                    statsg_n[n][:, 0:4], statsg_n[n][:, 4:8],
                    1.0 / 2048.0,
                    gpw_sb[:, n * 4:n * 4 + 4], bpw_sb[:, n * 4:n * 4 + 4],
                    scr, 4)
                for dc in range(4):
                    col = n * 4 + dc
                    sc = sc4[:, dc:dc + 1]
                    bi = bi4[:, dc:dc + 1]
                    rel = work.tile([128, 256], bf16 if BF16HT else f32,
                                    tag="rel", name=f"rel_{col}")
                    nc.scalar.activation(
                        rel[:], y_all[:, col * 256:(col + 1) * 256], Act.Relu,
                        bias=bi, scale=sc)
                    y2b = work.tile([128, 256], bf16, tag="y2",
                                    name=f"y2_{col}")
                    ht_src = htn_bf if BF16HT else htn_f32
                    nc.vector.tensor_add(
                        y2b[:], ht_src[:, col * 256:(col + 1) * 256], rel[:])
                    for b in range(2):
                        tp = psht.tile([128, 128], bf16, tag="ht",
                                       name=f"tp_{col}_{b}")
                        nc.tensor.transpose(
                            tp[:], y2b[:, b * 128:(b + 1) * 128], idn_sb[:])
                        dst = head2_sb[:, b * NH * DP + n * DP + dc * 128:
                                       b * NH * DP + n * DP + (dc + 1) * 128]
                        nc.vector.tensor_copy(dst, tp[:])

            # ---------- emission order ----------
            for n in range(NH):
                stage_a(n)
                stage_ar(n)
            for n in range(NH):
                stage_c(n)

            # ================= stage D: merge conv + stats ==================
            for b in range(2):
                m_ps = psht.tile([128, DP], f32, tag="ht", name=f"mps_{b}")
                for n in range(NH):
                    nc.tensor.matmul(
                        m_ps[:],
                        lhsT=wmt_sb[:, n * 128:(n + 1) * 128],
                        rhs=head2_sb[:, b * NH * DP + n * DP:
                                     b * NH * DP + (n + 1) * DP],
                        start=(n == 0), stop=(n == NH - 1),
                    )
                nc.vector.tensor_scalar(
                    m_sb[:, b * D:(b + 1) * D], m_ps[:, 0:D],
                    1.0, None, Alu.mult, op1=Alu.add,
                    accum_out=mstat[:, b:b + 1],
                )
                sq2 = work.tile([128, D], f32, tag="sq2", name=f"sqm_{b}")
                if SQ_DVE:
                    msb = m_sb[:, b * D:(b + 1) * D]
                    nc.vector.scalar_tensor_tensor(
                        out=sq2[:], in0=m_ps[:, 0:D], scalar=1.0, in1=msb,
                        op0=Alu.mult, op1=Alu.mult,
                        accum_out=mstat[:, 2 + b:3 + b],
                    )
                else:
                    nc.scalar.activation(
                        sq2[:], m_ps[:, 0:D], Act.Square,
                        accum_out=mstat[:, 2 + b:3 + b],
                    )
            nc.vector.tensor_add(mstat2[:, 0:1], mstat[:, 0:1], mstat[:, 1:2])
            nc.vector.tensor_add(mstat2[:, 1:2], mstat[:, 2:3], mstat[:, 3:4])

            # ================= stage E: AllReduce #2 + final BN =============
            ar2_in = dram.tile([128, 2], f32, name="ar2_in")
            ar2_out = dram.tile([128, 2], f32, addr_space="Shared",
                                name="ar2_out")
            nc.sync.dma_start(ar2_in[:], mstat2[:])
            nc.gpsimd.collective_compute(
                "AllReduce", Alu.add, replica_groups=rg,
                ins=[ar2_in[:].opt()], outs=[ar2_out[:].opt()],
            )
            statsg2 = const.tile([128, 2], f32)
            nc.gpsimd.dma_start(statsg2[:], ar2_out[:])

            scr2 = work.tile([128, 32], f32, tag="bnscr", bufs=2,
                             name="bnscr_m")
            sc2, bi2 = bn_coeffs(
                statsg2[:, 0:1], statsg2[:, 1:2], 1.0 / float(B * D),
                gmm_sb[:], bmm_sb[:], scr2, 1)
            for b in range(2):
                outf = work.tile([128, D], f32, tag="outf", name=f"outf_{b}")
                nc.scalar.activation(
                    outf[:], m_sb[:, b * D:(b + 1) * D], Act.Relu,
                    bias=bi2, scale=sc2)
                nc.sync.dma_start(out[:, b * D:(b + 1) * D], outf[:])

    nc.compile()
    return nc


def _pack_host(x):
    """Build per-core x-derived inputs from the full input."""
    xs = np.ascontiguousarray(x[..., 0].astype(np.float32))     # [B,C,H]

    xp = np.zeros((B, C, HB, 128), np.float32)
    xp[:, :, :, :HBS] = xs.reshape(B, C, HB, HBS)

    xt = np.zeros((B, HB, 128, 129), np.float32)
    xt[:, :, :HBS, :C] = xs.transpose(0, 2, 1).reshape(B, HB, HBS, C)
    xt[:, :, :HBS, C] = 1.0

    xpk_cores, xtg_cores = [], []
    for c in range(N_CORES):
        bs = [2 * c, 2 * c + 1]
        xpk_c = xp[bs].transpose(1, 0, 2, 3).reshape(128, 2 * HB * 128)
        xtg_c = xt[bs].transpose(2, 0, 1, 3).reshape(128, 2 * HB * 129)
        xpk_cores.append(np.ascontiguousarray(xpk_c.astype(BF)))
        xtg_cores.append(np.ascontiguousarray(xtg_c.astype(BF)))
    return xpk_cores, xtg_cores


def _pack_weights(W_pool, W_pw, g_pw, b_pw, W_merge, g_m, b_m):
    w = np.zeros((NH, 128, DP), np.float32)
    w[:, :, :D] = W_pool.transpose(0, 2, 1)
    wpt = w.transpose(1, 0, 2).reshape(128, NH * DP)

    Wp = np.zeros((NH, DP, DP), np.float32)
    Wp[:, :D, :D] = W_pw
    wpwt = (Wp.transpose(0, 2, 1).reshape(NH * 4, 128, DP)
            .transpose(1, 0, 2).reshape(128, NH * 4 * DP))

    wmt = (W_merge.reshape(CO, NH, 128).transpose(2, 1, 0)
           .reshape(128, NH * CO))

    G = np.zeros((NH, 4, 128), np.float32)
    G.reshape(NH, DP)[:, :D] = g_pw
    Bb = np.zeros((NH, 4, 128), np.float32)
    Bb.reshape(NH, DP)[:, :D] = b_pw
    gpw = np.ascontiguousarray(G.transpose(2, 0, 1).reshape(128, 16))
    bpw = np.ascontiguousarray(Bb.transpose(2, 0, 1).reshape(128, 16))

    return {
        "wpt": np.ascontiguousarray(wpt.astype(BF)),
        "wpwt": np.ascontiguousarray(wpwt.astype(BF)),
        "wmt": np.ascontiguousarray(wmt.astype(BF)),
        "gpw": gpw.astype(np.float32),
        "bpw": bpw.astype(np.float32),
        "gmm": np.ascontiguousarray(g_m[:, None].astype(np.float32)),
        "bmm": np.ascontiguousarray(b_m[:, None].astype(np.float32)),
        "idn": np.eye(128, dtype=BF),
    }


def make_in_maps(x, W_pool, W_pw, g_pw, b_pw, W_merge, g_m, b_m):
    xpk_cores, xtg_cores = _pack_host(np.asarray(x, dtype=np.float32))
    shared = _pack_weights(
        np.asarray(W_pool, np.float32), np.asarray(W_pw, np.float32),
        np.asarray(g_pw, np.float32), np.asarray(b_pw, np.float32),
        np.asarray(W_merge, np.float32), np.asarray(g_m, np.float32),
        np.asarray(b_m, np.float32))
    in_maps = []
    for c in range(N_CORES):
        m = dict(shared)
        m["xpk"] = xpk_cores[c]
        m["xtg"] = xtg_cores[c]
        in_maps.append(m)
    return in_maps


def gather_out(results):
    outs = np.empty((B, CO, D), np.float32)
    for c in range(N_CORES):
        o = results[c]["out"]
        outs[2 * c] = o[:, 0:D]
        outs[2 * c + 1] = o[:, D:2 * D]
    return outs[..., None]


def kernel(x, W_pool, W_pw, g_pw, b_pw, W_merge, g_m, b_m):
    nc = _build()
    from concourse import bass_utils
    in_maps = make_in_maps(x, W_pool, W_pw, g_pw, b_pw, W_merge, g_m, b_m)
    res = bass_utils.run_bass_kernel_spmd(
        nc, in_maps, core_ids=list(range(N_CORES)))
    return gather_out(res.results)


if __name__ == "__main__":
    rng = np.random.default_rng(0)
    ins = dict(
        x=rng.standard_normal((B, C, H, 1), dtype=np.float32),
        W_pool=(rng.standard_normal((NH, D, C)) * 0.05).astype(np.float32),
        W_pw=(rng.standard_normal((NH, D, D)) * 0.05).astype(np.float32),
        g_pw=np.ones((NH, D), np.float32),
        b_pw=np.zeros((NH, D), np.float32),
        W_merge=(rng.standard_normal((CO, NH * C)) * 0.02).astype(np.float32),
        g_m=np.ones((CO,), np.float32),
        b_m=np.zeros((CO,), np.float32),
    )
    print(kernel(**ins).shape)
